# revision 22
# baseline (speedup 1.0000x reference)
"""DifferentiableMatcher Trainium2 kernel.

cost[k, n] = 1 - <pred_k, gt_n> over HW=512*512, then 5 Sinkhorn iterations
(row/col logsumexp normalizations) and exp.

Strategy (8 NeuronCores):
  - Shard the HW contraction: core c owns HW slice [c*32768, (c+1)*32768).
  - Inputs cast to fp16 on host (exact fp32 PSUM accumulation; rel err ~6e-4)
    halving HBM traffic. Host packs each shard so SBUF partition p holds runs
    of FB=4 HW elements per (q, row): packed[c,b,p,(q,row,f)] = x[row, h],
    h = c*32768 + b*8192 + q*512 + p*4 + f. DMA per partition is contiguous.
  - Per core: 256 accumulating fp16 matmuls with PT stationary ([128,100]
    weights, GT streaming 50 cols; 93ns/pair with LDWEIGHTS pipelined) ->
    partial dot [K=100, N=50] in PSUM. The first DMA block is split into
    small pieces so matmuls start ~10.6us in; the last block is halved so
    the tail matmuls (and the AllReduce trigger) finish sooner.
  - Cross-core reduce: 20KB fp32 AllReduce via the ncfw mesh collective,
    then Sinkhorn runs replicated. (A hand-rolled remote_dma_broadcast
    reduce -- kept under DM_USE_COLLECTIVE=0 -- is functionally correct but
    the remote-DMA data path takes ~2.5-13ms per transfer in this
    environment, so the collective is the default.)
  - Sinkhorn as diagonal scaling: after the first row normalization
    (log-space, max-subtracted, via one exp) the iteration is
    V = 1/(E''^T U), U = 1/(E'' V) with E'' = exp(l - rowmax - colmax)
    (the two max shifts cancel exactly in the row/col normalizations, and
    make every row and column of E'' contain a 1.0 so no sum can
    under/overflow). Each half-step is one tiny PE matvec + one DVE
    reciprocal; final output = diag(U) E'' diag(V) materialized via a
    PE broadcast matmul. ~10us vs ~20us for the log-space version.
  - The exp activation table is prefetched by a dummy activation at kernel
    start so the first Sinkhorn exp doesn't pay the ~1.3us table load.
"""

import os

import numpy as np

K = 100
N = 50
HW = 512 * 512
CORES = 8
SHARD = HW // CORES  # 32768
P = 128
FB = 4
Q = SHARD // (P * FB)  # 64 q-steps per core
NBLK = 8
QB = Q // NBLK  # 8 q-steps per DMA block
B0_PIECES = [1, 1, 2, 4]  # first block split for fast pipeline start
TEMP = 0.1
ITERS = 5

USE_COLLECTIVE = bool(int(os.environ.get("DM_USE_COLLECTIVE", "1")))
USE_ALLGATHER = bool(int(os.environ.get("DM_USE_ALLGATHER", "1")))

_CACHE = {}

TRACE = False
TRACE_KW = {}
LAST_RESULT = None


def _patch_act_tables():
    """Make the combined Exp+Ln table set the only candidate for Exp/Ln so
    the compiler emits one table load instead of thrashing per activation."""
    import concourse.hw_specs as hw_specs
    from concourse import bacc as bacc_mod
    from concourse import mybir

    if getattr(bacc_mod, "_act_tables_patched", False):
        return
    orig = hw_specs.get_activation_tables

    def patched(arch):
        t = orig(arch)
        exp = mybir.ActivationFunctionType.Exp
        ln = mybir.ActivationFunctionType.Ln
        out = {}
        for name, funcs in t.items():
            if (exp in funcs) != (ln in funcs):
                funcs = funcs - {exp, ln}
            out[name] = funcs
        return out

    bacc_mod.get_activation_tables = patched
    bacc_mod._act_tables_patched = True


def _build():
    from concourse import bacc, tile, mybir
    from concourse.masks import make_identity
    from concourse.tile import add_dep_helper

    _patch_act_tables()

    f16 = mybir.dt.float16
    f32 = mybir.dt.float32
    bf = mybir.dt.bfloat16
    Exp = mybir.ActivationFunctionType.Exp
    X = mybir.AxisListType.X
    mult = mybir.AluOpType.mult
    add = mybir.AluOpType.add

    nc = bacc.Bacc(
        "TRN2",
        target_bir_lowering=False,
        debug=False,
        enable_asserts=False,
        num_devices=CORES,
        monotonic_sem_count=0,
    )
    p_in = nc.dram_tensor(
        "p_in", [NBLK, P, QB * K * FB], f16, kind="ExternalInput"
    ).ap()
    g_in = nc.dram_tensor(
        "g_in", [NBLK, P, QB * N * FB], f16, kind="ExternalInput"
    ).ap()
    out = nc.dram_tensor("out", [K, N], f32, kind="ExternalOutput").ap()

    rsem = nc.alloc_semaphore("rdma_remote")
    lsem = nc.alloc_semaphore("rdma_local")

    with tile.TileContext(nc) as tc:
        with (
            tc.tile_pool(name="pp", bufs=NBLK) as pp,
            tc.tile_pool(name="gp", bufs=NBLK) as gp,
            tc.tile_pool(name="sk", bufs=1) as sk,
            tc.tile_pool(name="cps", bufs=1, space="PSUM") as cps,
            tc.tile_pool(name="tps", bufs=1, space="PSUM") as tps,
            tc.tile_pool(name="dram", bufs=1, space="DRAM") as dram,
        ):
            # --- early, off critical path: exp table prefetch, consts ---
            warm = sk.tile([1, 1], f32)
            nc.gpsimd.memset(warm, 0.0)
            warm2 = sk.tile([1, 1], f32)
            nc.scalar.activation(out=warm2, in_=warm, func=Exp)
            ident = sk.tile([P, P], f32)
            make_identity(nc, ident)
            identb = sk.tile([P, P], bf)
            make_identity(nc, identb)
            ones1K = sk.tile([1, K], bf)
            nc.gpsimd.memset(ones1K, 1.0)

            # receive buffer for the cross-core reduce: 8 slots of [*, N]
            recv = sk.tile([P, CORES * N], f32)

            if not USE_COLLECTIVE:
                # prepare the 7 remote sends early (descriptor gen only);
                # slot j on the receiver (tpb XOR j) gets our slot 0
                for j in range(1, CORES):
                    rdests = [None] * CORES
                    rdests[j] = (0, j)
                    nc.gpsimd.remote_dma_broadcast(
                        out_ap=recv[:, j * N : (j + 1) * N],
                        in_ap=recv[:, 0:N],
                        remote_sem=rsem,
                        local_sem=lsem,
                        rdests=rdests,
                    )

            # --- main contraction: 256 accumulating matmuls ---
            # PT stationary ([128,100] weights), GT moving (50 cols): the
            # LDWEIGHTS/MATMUL pair pipelines to ~93ns; the swapped order
            # measured slower (135ns/pair, no LDW overlap).
            C = cps.tile([K, N], f32)
            first = True
            for b in range(NBLK):
                PT = pp.tile([P, QB * K * FB], f16)
                GT = gp.tile([P, QB * N * FB], f16)
                if b == 0:
                    off = 0
                    for plen in B0_PIECES:
                        nc.scalar.dma_start(
                            out=GT[:, off * N * FB : (off + plen) * N * FB],
                            in_=g_in[0][:, off * N * FB : (off + plen) * N * FB],
                        )
                        nc.sync.dma_start(
                            out=PT[:, off * K * FB : (off + plen) * K * FB],
                            in_=p_in[0][:, off * K * FB : (off + plen) * K * FB],
                        )
                        off += plen
                elif b == NBLK - 1:
                    half = QB // 2
                    for off, plen in ((0, half), (half, QB - half)):
                        nc.sync.dma_start(
                            out=GT[:, off * N * FB : (off + plen) * N * FB],
                            in_=g_in[b][:, off * N * FB : (off + plen) * N * FB],
                        )
                        nc.scalar.dma_start(
                            out=PT[:, off * K * FB : (off + plen) * K * FB],
                            in_=p_in[b][:, off * K * FB : (off + plen) * K * FB],
                        )
                elif b % 2 == 0:
                    nc.scalar.dma_start(out=GT, in_=g_in[b])
                    nc.sync.dma_start(out=PT, in_=p_in[b])
                else:
                    nc.sync.dma_start(out=GT, in_=g_in[b])
                    nc.scalar.dma_start(out=PT, in_=p_in[b])
                PT4 = PT.rearrange("p (q k f) -> p q k f", k=K, f=FB)
                GT4 = GT.rearrange("p (q n f) -> p q n f", n=N, f=FB)
                for q in range(QB):
                    for f in range(FB):
                        nc.tensor.matmul(
                            C,
                            PT4[:, q, :, f],
                            GT4[:, q, :, f],
                            start=first,
                            stop=(b == NBLK - 1 and q == QB - 1 and f == FB - 1),
                        )
                        first = False

            if USE_COLLECTIVE and USE_ALLGATHER:
                c_sb = sk.tile([K, N], f32)
                nc.vector.tensor_scalar(
                    out=c_sb, in0=C, scalar1=1.0 / TEMP, scalar2=None, op0=mult
                )
                din = dram.tile([K, N], f32)
                nc.sync.dma_start(out=din[0 : K // 2], in_=c_sb[0 : K // 2])
                nc.scalar.dma_start(out=din[K // 2 : K], in_=c_sb[K // 2 : K])
                doutg = dram.tile([CORES * K, N], f32, addr_space="Shared")
                nc.gpsimd.collective_compute(
                    "AllGather",
                    mybir.AluOpType.bypass,
                    replica_groups=[list(range(CORES))],
                    ins=[din.opt()],
                    outs=[doutg.opt()],
                )
                dgv = doutg.rearrange("(c k) n -> k c n", c=CORES)
                rc3 = recv[0:K, :].rearrange("k (c n) -> k c n", c=CORES)
                with nc.allow_non_contiguous_dma("160KB one-shot gather"):
                    nc.sync.dma_start(out=rc3, in_=dgv)
                s1 = sk.tile([K, 4 * N], f32)
                nc.vector.tensor_add(
                    s1, recv[0:K, 0 : 4 * N], recv[0:K, 4 * N : 8 * N]
                )
                s2 = sk.tile([K, 2 * N], f32)
                nc.vector.tensor_add(s2, s1[:, 0 : 2 * N], s1[:, 2 * N : 4 * N])
                csum = sk.tile([K, N], f32)
                nc.vector.tensor_add(csum, s2[:, 0:N], s2[:, N : 2 * N])
            elif USE_COLLECTIVE:
                c_sb = sk.tile([K, N], f32)
                nc.vector.tensor_scalar(
                    out=c_sb, in0=C, scalar1=1.0 / TEMP, scalar2=None, op0=mult
                )
                din = dram.tile([K, N], f32)
                dout = dram.tile([K, N], f32, addr_space="Shared")
                nc.sync.dma_start(out=din, in_=c_sb)
                nc.gpsimd.collective_compute(
                    "AllReduce",
                    mybir.AluOpType.add,
                    replica_groups=[list(range(CORES))],
                    ins=[din.opt()],
                    outs=[dout.opt()],
                )
                csum = sk.tile([K, N], f32)
                nc.sync.dma_start(out=csum, in_=dout)
            else:
                # combined raw partial into recv slot 0 (the send source);
                # the 1/TEMP scale is applied after the tree reduce
                ts = nc.vector.tensor_scalar(
                    out=recv[0:K, 0:N], in0=C, scalar1=1.0, scalar2=None, op0=mult
                )
                trig = nc.gpsimd.trigger_dma(count=None)
                add_dep_helper(trig.ins, ts.ins, reason="partial ready before send")
                # Trace a trivially-satisfied wait (the Tile scheduler's
                # single-core sim cannot model the remote increments); the
                # real >=14 arrival barrier is appended post-schedule below.
                w = nc.vector.wait_ge(rsem, 0)
                add_dep_helper(w.ins, ts.ins, sync=False, reason="wait after ts")
                s1 = sk.tile([K, 4 * N], f32)
                a1 = nc.vector.tensor_add(
                    s1, recv[0:K, 0 : 4 * N], recv[0:K, 4 * N : 8 * N]
                )
                add_dep_helper(a1.ins, w.ins, sync=False, reason="recv barrier")
                s2 = sk.tile([K, 2 * N], f32)
                nc.vector.tensor_add(s2, s1[:, 0 : 2 * N], s1[:, 2 * N : 4 * N])
                craw = sk.tile([K, N], f32)
                nc.vector.tensor_add(craw, s2[:, 0:N], s2[:, N : 2 * N])
                csum = sk.tile([K, N], f32)
                nc.vector.tensor_scalar(
                    out=csum, in0=craw, scalar1=1.0 / TEMP, scalar2=None, op0=mult
                )

            # --- Sinkhorn: setup (first row-norm + stabilized E'' / E''^T) ---
            nM = sk.tile([K, 1], f32)
            nc.vector.reduce_max(out=nM, in_=csum, axis=X, negate=True)
            Escr = sk.tile([K, N], f32)
            S = sk.tile([K, 1], f32)
            nc.scalar.activation(out=Escr, in_=csum, func=Exp, bias=nM, accum_out=S)
            U = sk.tile([K, 1], bf, tag="U")
            with nc.allow_low_precision("bf16 sinkhorn scales, validated offline"):
                nc.vector.reciprocal(U, S)
            lpp = sk.tile([K, N], f32)
            nc.vector.tensor_scalar(
                out=lpp, in0=csum, scalar1=nM, scalar2=None, op0=add
            )
            T1 = tps.tile([N, K], f32, tag="T1")
            nc.tensor.transpose(T1, lpp, ident[:K, :K])  # [N,K] in PSUM
            nMT = sk.tile([N, 1], f32)
            nc.vector.reduce_max(out=nMT, in_=T1, axis=X, negate=True)
            ET = sk.tile([N, K], bf)
            nc.scalar.activation(out=ET, in_=T1, func=Exp, bias=nMT)
            EppP = tps.tile([K, N], bf, tag="T0")
            nc.tensor.transpose(EppP, ET, identb[:N, :N])
            Epp = sk.tile([K, N], bf)
            nc.vector.tensor_copy(Epp, EppP)

            # --- 9 half-steps: V,U,V,U,V,U,V,U,V ---
            Vp = None
            for step in range(2 * ITERS - 1):
                last = step == 2 * ITERS - 2
                if step % 2 == 0:
                    if last:
                        # final V as a row [1,N] for the broadcast matmul
                        cs = tps.tile([1, N], f32, tag="cs")
                        nc.tensor.matmul(cs, U, Epp, start=True, stop=True)
                        Vp = sk.tile([1, N], bf, tag="Vp")
                        with nc.allow_low_precision("bf16 sinkhorn scales"):
                            nc.vector.reciprocal(Vp, cs)
                    else:
                        # colsum directly as a column [N,1]: mm(lhsT=Epp, U)
                        csT = tps.tile([N, 1], f32, tag="csT")
                        nc.tensor.matmul(csT, Epp, U, start=True, stop=True)
                        Vc = sk.tile([N, 1], bf, tag="Vc")
                        with nc.allow_low_precision("bf16 sinkhorn scales"):
                            nc.vector.reciprocal(Vc, csT)
                else:
                    rs = tps.tile([K, 1], f32, tag="rs")
                    nc.tensor.matmul(rs, ET, Vc, start=True, stop=True)
                    U = sk.tile([K, 1], bf, tag="U")
                    with nc.allow_low_precision("bf16 sinkhorn scales"):
                        nc.vector.reciprocal(U, rs)
                    if step == 2 * ITERS - 3:
                        # fp32 copy of the final U for the materialization
                        U32 = sk.tile([K, 1], f32, tag="U32")
                        nc.vector.reciprocal(U32, rs)

            # --- final: out = diag(U) E'' diag(V) ---
            Y = sk.tile([K, N], f32)
            nc.vector.tensor_scalar(
                out=Y, in0=Epp, scalar1=U32, scalar2=None, op0=mult
            )
            B = tps.tile([K, N], f32, tag="B")
            nc.tensor.matmul(B, ones1K, Vp, start=True, stop=True)
            res = sk.tile([K, N], f32)
            nc.vector.tensor_mul(res, Y, B)
            nc.sync.dma_start(out=out, in_=res)

    if not USE_COLLECTIVE:
        # Append the real arrival barrier now that scheduling is done: the
        # vector engine blocks until all 7 peers' partials have landed
        # (each remote send increments rsem by 2 on arrival).
        w.wait_op(rsem, 2 * (CORES - 1), "sem-ge", check=False)
    nc.compile()
    return nc


def _get_nc():
    if "nc" not in _CACHE:
        _CACHE["nc"] = _build()
    return _CACHE["nc"]


def _get_runner():
    """Cached PJRT executable (mirrors bass2jax.run_bass_via_pjrt's multi-core
    branch) so repeat kernel() calls skip retracing/recompiling."""
    if "runner" in _CACHE:
        return _CACHE["runner"]
    import jax
    from jax.experimental.shard_map import shard_map
    from jax.sharding import Mesh, PartitionSpec

    from concourse import bass2jax, mybir

    nc = _get_nc()
    bass2jax.install_neuronx_cc_hook()
    assert nc.dbg_addr is None
    partition_name = nc.partition_id_tensor.name if nc.partition_id_tensor else None

    in_names, out_names, out_avals, out_shapes = [], [], [], []
    for alloc in nc.m.functions[0].allocations:
        if not isinstance(alloc, mybir.MemoryLocationSet):
            continue
        name = alloc.memorylocations[0].name
        if alloc.kind == "ExternalInput":
            if name != partition_name:
                in_names.append(name)
        elif alloc.kind == "ExternalOutput":
            shape = tuple(alloc.tensor_shape)
            dtype = mybir.dt.np(alloc.dtype)
            out_avals.append(jax.core.ShapedArray(shape, dtype))
            out_shapes.append((name, shape, dtype))
            out_names.append(name)
    n_params = len(in_names)
    n_outs = len(out_names)
    all_in_names = list(in_names) + list(out_names)
    if partition_name is not None:
        all_in_names.append(partition_name)
    donate = tuple(range(n_params, n_params + n_outs))

    def _body(*args):
        operands = list(args)
        if partition_name is not None:
            operands.append(bass2jax.partition_id_tensor())
        outs = bass2jax._bass_exec_p.bind(
            *operands,
            out_avals=tuple(out_avals),
            in_names=tuple(all_in_names),
            out_names=tuple(out_names),
            lowering_input_output_aliases=(),
            sim_require_finite=True,
            sim_require_nnan=True,
            nc=nc,
        )
        return tuple(outs)

    devices = jax.devices()[:CORES]
    mesh = Mesh(np.asarray(devices), ("core",))
    in_specs = (PartitionSpec("core"),) * (n_params + n_outs)
    out_specs = (PartitionSpec("core"),) * n_outs
    sharded = jax.jit(
        shard_map(
            _body, mesh=mesh, in_specs=in_specs, out_specs=out_specs, check_rep=False
        ),
        donate_argnums=donate,
        keep_unused=True,
    )
    _CACHE["runner"] = (sharded, in_names, out_shapes)
    return _CACHE["runner"]


def _pack(arr, rows):
    # arr [rows, HW] fp32 -> [CORES, NBLK, P, QB*rows*FB] fp16, with
    # packed[c, b, p, (q, r, f)] = arr[r, c*SHARD + b*QB*512 + q*512 + p*FB + f]
    v = arr.reshape(rows, CORES, NBLK, QB, P, FB).transpose(1, 2, 4, 3, 0, 5)
    return v.astype(np.float16).reshape(CORES, NBLK, P, QB * rows * FB)


def kernel(pred_masks, gt_masks):
    global LAST_RESULT
    from concourse import bass_utils

    pred = np.ascontiguousarray(np.asarray(pred_masks, dtype=np.float32)).reshape(
        K, HW
    )
    gt = np.ascontiguousarray(np.asarray(gt_masks, dtype=np.float32)).reshape(N, HW)
    pk = _pack(pred, K)
    gk = _pack(gt, N)
    in_maps = [{"p_in": pk[c], "g_in": gk[c]} for c in range(CORES)]
    if TRACE:
        nc = _get_nc()
        res = bass_utils.run_bass_kernel_spmd(
            nc, in_maps, core_ids=list(range(CORES)), trace=TRACE, **TRACE_KW
        )
        LAST_RESULT = res
        o = np.asarray(res.results[0]["out"], dtype=np.float32)
        return np.ascontiguousarray(o).reshape(1, K, N)

    sharded, in_names, out_shapes = _get_runner()
    concat_in = [
        np.concatenate([in_maps[c][name] for c in range(CORES)], axis=0)
        for name in in_names
    ]
    concat_zeros = [
        np.zeros((CORES * shape[0], *shape[1:]), dtype) for _, shape, dtype in out_shapes
    ]
    out_arrs = sharded(*concat_in, *concat_zeros)
    out0 = np.asarray(out_arrs[0]).reshape(CORES, K, N)[0]
    return np.ascontiguousarray(out0.astype(np.float32)).reshape(1, K, N)
